# revision 2
# baseline (speedup 1.0000x reference)
"""Tensor-parallel causal attention layer (RoPE) for 8 Trainium2 NeuronCores.

Problem: nn_AttentionTier (B=4, T=2048, D=1024, H=16, Dh=64), fp32 I/O.

Sharding: DP=4 over batch x TP=2 over heads (8 heads per core).
  core c -> batch c//2, head group c%2 (heads 8*(c%2) .. 8*(c%2)+8).

v2 design:
  - All on-chip tensors bf16 (host pre-casts inputs): half DMA traffic, DVE
    2x perf mode, no fp32r small-moving matmul penalty.
  - Projection (PE-heavy, ACT-idle) and attention (ACT-heavy) INTERLEAVED
    per 512-token block: proj(tb) ; attn(qb=tb). The softmax exp stream for
    block qb overlaps the projection matmuls for block tb=qb+1.
  - Causal mask applied INSIDE the PE accumulation: after the diagonal score
    matmul, a second matmul (identity lhsT x (-240 strict-upper) rhs)
    accumulates -240 into masked entries, so exp(0.125*s) underflows to 0.
    No DVE op between exp and the AV matmul.
  - The per-qb out-proj ReduceScatter collective of the baseline is replaced
    by ONE SBUF->SBUF pairwise exchange of normalized attention outputs
    (remote_dma_broadcast, relative dest (0,1) = pair core), then each core
    computes the full out-projection for its own 512 output features over
    all 16 heads. woutT rows host-reordered [my feats; peer feats].
  - Softmax denominators: o_ps row 64 (ones-augmented V) -> per-qb [65,H,TB]
    evac tile; single DMA gathers all 8 sumexp rows (bf16->f32 cast) into a
    [1, H*TB] stack; one DVE reciprocal + bf16 cast; per-head K=1 matmul
    broadcasts recips over 64 partitions; one DVE mult normalizes.
  - Weights DMA'd before x so the PE starts ~immediately.
"""

import sys

sys.path.insert(0, "/opt/trn_rl_repo")

import numpy as np

B, T, D = 4, 2048, 1024
H, Dh = 16, 64
N_CORES = 8
P = 128
TB = 512          # token block (matmul moving dim)
NTB = T // TB     # 4
NCC = D // P      # 8 contraction chunks
HLOC = H // 2     # heads per core

_CACHE = {}


def _build_program(reps=1, exch="rdma"):
    import concourse.bass as bass  # noqa: F401
    import concourse.mybir as mybir
    import concourse.tile as tile
    from concourse import bacc

    f32 = mybir.dt.float32
    bf16 = mybir.dt.bfloat16
    AF = mybir.ActivationFunctionType

    nc = bacc.Bacc("TRN2", target_bir_lowering=False, debug=False,
                   num_devices=N_CORES)

    # ---- DRAM I/O (bf16 in/out; host casts) ----
    xT_d = nc.dram_tensor("xT", [D, T], bf16, kind="ExternalInput").ap()
    wqkT_d = nc.dram_tensor("wqkT", [D, D], bf16, kind="ExternalInput").ap()
    wvT_d = nc.dram_tensor("wvT", [D, D // 2], bf16, kind="ExternalInput").ap()
    woutT_d = nc.dram_tensor("woutT", [D, D // 2], bf16,
                             kind="ExternalInput").ap()
    r2T_d = nc.dram_tensor("r2T", [P, P], bf16, kind="ExternalInput").ap()
    cos2_d = nc.dram_tensor("cos2", [P, T], bf16, kind="ExternalInput").ap()
    sin2_d = nc.dram_tensor("sin2", [P, T], bf16, kind="ExternalInput").ap()
    mneg_d = nc.dram_tensor("mneg", [P, P], bf16, kind="ExternalInput").ap()
    ident_d = nc.dram_tensor("ident", [P, P], bf16, kind="ExternalInput").ap()
    out_d = nc.dram_tensor("out", [D // 2, T], bf16, kind="ExternalOutput").ap()

    # exchange semaphores (SPMD: same numbers on all cores)
    prep = nc.alloc_semaphore("prep")
    lsem = nc.alloc_semaphore("lsem")
    rsem = nc.alloc_semaphore("rsem")

    with tile.TileContext(nc) as tc:
        with tc.tile_pool(name="const", bufs=1) as constp, \
             tc.tile_pool(name="big", bufs=1) as bigp:

            r2T = constp.tile([P, P], bf16)
            nc.sync.dma_start(r2T[:], r2T_d[:])
            mneg = constp.tile([P, P], bf16)
            nc.sync.dma_start(mneg[:], mneg_d[:])
            ident = constp.tile([P, P], bf16)
            nc.sync.dma_start(ident[:], ident_d[:])
            ones_b = constp.tile([P, P], bf16)
            nc.vector.memset(ones_b[:], 1.0)

            # persistent big tensors
            qk = bigp.tile([P, NCC, T], bf16)                    # 32KB/p
            vbar = bigp.tile([P, T // P, HLOC, Dh + 1], bf16)    # ~17KB/p
            aout = bigp.tile([P, NTB, NCC // 2, TB], bf16)       # 16KB/p
            aout_peer = bigp.tile([P, NTB, NCC // 2, TB], bf16)  # 16KB/p

            def body():
                with tc.tile_pool(name="w1", bufs=1) as w1p, \
                     tc.tile_pool(name="ph1", bufs=3) as ph1, \
                     tc.tile_pool(name="xtp", bufs=2) as xtp, \
                     tc.tile_pool(name="att", bufs=6) as attp, \
                     tc.tile_pool(name="msc", bufs=2) as mscp, \
                     tc.tile_pool(name="msc1", bufs=1) as mscp1, \
                     tc.tile_pool(name="ps_a", bufs=2, space="PSUM") as ps_a, \
                     tc.tile_pool(name="ps_b", bufs=2, space="PSUM") as ps_b:
                    # psum tags: "qs" qk-proj/rot (2 banks), "sps" scores
                    # (2x2 banks), "vob" v-proj/o/b (2 banks) => 8 banks
                    wqkT = w1p.tile([P, NCC, D], bf16)
                    wvT = w1p.tile([P, NCC, D // 2], bf16)
                    woutT = w1p.tile([P, NCC, D // 2], bf16)
                    cosb = w1p.tile([P, T], bf16)
                    sinb = w1p.tile([P, T], bf16)
                    # first weight chunk first, then x block 0, then the rest
                    nc.sync.dma_start(wqkT[:, 0], wqkT_d[0:P, :])

                    def load_xT(tb):
                        t = xtp.tile([P, NCC, TB], bf16, tag="xT")
                        for cc in range(NCC):
                            nc.sync.dma_start(
                                t[:, cc],
                                xT_d[cc * P:(cc + 1) * P,
                                     tb * TB:(tb + 1) * TB])
                        return t

                    xT0 = xtp.tile([P, NCC, TB], bf16, tag="xT")
                    nc.sync.dma_start(xT0[:, 0], xT_d[0:P, 0:TB])
                    for cc in range(1, NCC):
                        nc.sync.dma_start(
                            wqkT[:, cc], wqkT_d[cc * P:(cc + 1) * P, :])
                        nc.sync.dma_start(
                            xT0[:, cc], xT_d[cc * P:(cc + 1) * P, 0:TB])
                    for cc in range(NCC):
                        nc.sync.dma_start(
                            wvT[:, cc], wvT_d[cc * P:(cc + 1) * P, :])
                    nc.sync.dma_start(cosb[:], cos2_d[:])
                    nc.sync.dma_start(sinb[:], sin2_d[:])
                    nc.sync.dma_start(
                        woutT[:], woutT_d.rearrange("(cc p) o -> p cc o", p=P))

                    def rope_tail(oc, raw, tsl):
                        """rot matmul + cos/sin combine for chunk oc."""
                        rot_ps = ps_a.tile([P, TB], f32, tag="qs",
                                           name=f"rot_{oc}_{tsl.start}")
                        nc.tensor.matmul(rot_ps[:], r2T[:], raw[:],
                                         start=True, stop=True)
                        m1 = ph1.tile([P, TB], bf16, tag="m1")
                        nc.vector.tensor_tensor(
                            m1[:], raw[:], cosb[:, tsl],
                            mybir.AluOpType.mult)
                        m2 = ph1.tile([P, TB], bf16, tag="m2")
                        nc.vector.tensor_tensor(
                            m2[:], rot_ps[:], sinb[:, tsl],
                            mybir.AluOpType.mult)
                        nc.vector.tensor_tensor(
                            qk[:, oc, tsl], m1[:], m2[:],
                            mybir.AluOpType.add)

                    def proj(tb):
                        tsl = slice(tb * TB, (tb + 1) * TB)
                        xT = xT0 if tb == 0 else load_xT(tb)
                        pend = None
                        for oc in range(NCC):
                            qk_ps = ps_a.tile([P, TB], f32, tag="qs",
                                              name=f"qk_{oc}_{tb}")
                            for cc in range(NCC):
                                nc.tensor.matmul(
                                    qk_ps[:], wqkT[:, cc, oc * P:(oc + 1) * P],
                                    xT[:, cc, :],
                                    start=(cc == 0), stop=(cc == NCC - 1))
                            raw = ph1.tile([P, TB], bf16, tag="raw")
                            nc.vector.tensor_copy(raw[:], qk_ps[:])
                            if pend is not None:
                                rope_tail(pend[0], pend[1], tsl)
                            pend = (oc, raw)

                        # V projection (natural layout), rope tail of the
                        # last chunk slotted after the first V block
                        for ts in range(TB // P):
                            v_ps = ps_b.tile([P, D // 2], f32, tag="vob",
                                             name=f"v_{tb}_{ts}")
                            for cc in range(NCC):
                                nc.tensor.matmul(
                                    v_ps[:], xT[:, cc, ts * P:(ts + 1) * P],
                                    wvT[:, cc, :],
                                    start=(cc == 0), stop=(cc == NCC - 1))
                            tc_idx = tb * (TB // P) + ts
                            nc.vector.tensor_copy(
                                vbar[:, tc_idx, :, 0:Dh],
                                v_ps[:].rearrange("p (h d) -> p h d", h=HLOC))
                            if ts == 0:
                                rope_tail(pend[0], pend[1], tsl)
                                pend = None
                        # ones column for this tb's token chunks
                        nc.vector.tensor_copy(
                            vbar[:, 4 * tb:4 * tb + 4, :, Dh:Dh + 1],
                            ones_b[:, None, :HLOC, None].to_broadcast(
                                [P, 4, HLOC, 1]))

                    def attn(qb):
                        osball = mscp.tile([Dh + 1, HLOC, TB], bf16,
                                           tag="osball")
                        # sumexp rows live at partitions {0,32} x 4 cols so
                        # K=1 broadcast matmuls see 32-aligned bases.
                        # Gathered via HWDGE (bf16) to keep the SWDGE ring
                        # exclusively for the remote exchange.
                        sstack_b = mscp1.tile([P, 4, TB], bf16, tag="sstack_b")
                        for h in range(HLOC):
                            hb = Dh * (h % 2)
                            # q feats: chunks 0..3; k feats: chunks 4..7
                            qsl = (slice(hb, hb + Dh), h // 2,
                                   slice(qb * TB, (qb + 1) * TB))
                            ksl = lambda kc: qk[hb:hb + Dh, NCC // 2 + h // 2,
                                                kc * P:(kc + 1) * P]
                            o_ps = ps_b.tile([Dh + 1, TB], f32, tag="vob",
                                             name=f"o_{qb}_{h}")
                            # full (off-diagonal) k-chunks, two per exp
                            for kp in range(2 * qb):
                                k0 = 2 * kp
                                s_ps = ps_a.tile([P, 2, TB], f32, tag="sps")
                                nc.tensor.matmul(
                                    s_ps[:, 0, :], ksl(k0), qk[qsl],
                                    start=True, stop=True)
                                nc.tensor.matmul(
                                    s_ps[:, 1, :], ksl(k0 + 1), qk[qsl],
                                    start=True, stop=True)
                                pt = attp.tile([P, 2, TB], bf16, tag="pt")
                                nc.scalar.activation(
                                    pt[:], s_ps[:], AF.Exp, scale=0.125)
                                for j in range(2):
                                    nc.tensor.matmul(
                                        o_ps[:], vbar[:, k0 + j, h, :],
                                        pt[:, j, :],
                                        start=(k0 + j == 0), stop=False,
                                        skip_group_check=True)
                            # diagonal k-chunks; causal mask folded into the
                            # PE accumulation (-240 on strict upper triangle)
                            for cr in range(4):
                                kc = 4 * qb + cr
                                qo = cr * P
                                s_ps = ps_a.tile([P, 2, TB], f32, tag="sps")
                                nc.tensor.matmul(
                                    s_ps[:, 0, qo:TB], ksl(kc),
                                    qk[qsl][:, qo:TB],
                                    start=True, stop=False)
                                nc.tensor.matmul(
                                    s_ps[:, 0, qo:qo + P], ident[:], mneg[:],
                                    start=False, stop=True,
                                    skip_group_check=True)
                                pt = attp.tile([P, 2, TB], bf16, tag="pt")
                                nc.scalar.activation(
                                    pt[:, 0, qo:TB], s_ps[:, 0, qo:TB], AF.Exp,
                                    scale=0.125)
                                nc.tensor.matmul(
                                    o_ps[:, qo:TB], vbar[:, kc, h, :],
                                    pt[:, 0, qo:TB],
                                    start=(kc == 0), stop=(kc == 4 * qb + 3),
                                    skip_group_check=True)
                            # evacuate o_ps (incl. sumexp row 64) to bf16
                            nc.vector.tensor_copy(osball[:, h, :], o_ps[:])
                        # gather sumexp rows: head h lands at partition
                        # 32*(h//4), col h%4
                        for g in range(2):
                            nc.sync.dma_start(
                                sstack_b[32 * g:32 * g + 1, :, :],
                                osball[Dh:Dh + 1, 4 * g:4 * g + 4, :])
                        sstack = mscp1.tile([P, 4, TB], f32, tag="sstack")
                        rstack = mscp.tile([P, 4, TB], bf16, tag="rstack")
                        nc.vector.tensor_copy(sstack[:], sstack_b[:])
                        nc.vector.reciprocal(sstack[:], sstack[:])
                        nc.vector.tensor_copy(rstack[:], sstack[:])
                        return osball, rstack

                    def finish_attn(qb, osball, rstack):
                        """Recip broadcast + normalize; emitted after the
                        NEXT proj block so the b_ps matmuls never stall the
                        PE queue on the reciprocal chain."""
                        for h in range(HLOC):
                            hb = Dh * (h % 2)
                            rrow = 32 * (h // 4)
                            b_ps = ps_b.tile([Dh + 1, TB], f32, tag="vob",
                                             name=f"b_{qb}_{h}")[0:Dh]
                            nc.tensor.matmul(
                                b_ps[:], ones_b[rrow:rrow + 1, 0:Dh],
                                rstack[rrow:rrow + 1, h % 4, :],
                                start=True, stop=True)
                            nc.vector.tensor_tensor(
                                aout[hb:hb + Dh, qb, h // 2, :],
                                osball[0:Dh, h, :], b_ps[:],
                                mybir.AluOpType.mult)

                    pend_fin = None
                    for tb in range(NTB):
                        proj(tb)
                        if pend_fin is not None:
                            finish_attn(*pend_fin)
                        pend_fin = (tb,) + attn(tb)
                    finish_attn(*pend_fin)

                    # ======== exchange + out-projection ========
                    if exch == "rdma":
                        with tc.tile_critical():
                            for i in range(NTB):
                                nc.gpsimd.remote_dma_broadcast(
                                    aout_peer[:, i], aout[:, i], rsem, lsem,
                                    rdests=[(0, 1)] * 8).then_inc(prep, 1)
                            nc.gpsimd.wait_ge(prep, NTB)
                            nc.gpsimd.trigger_dma(NTB)
                            nc.gpsimd.wait_ge(rsem, 16 * NTB)
                            nc.gpsimd.wait_ge(lsem, 16 * NTB)
                            if reps > 1:
                                # reset for the next rep; peer is >200us from
                                # its next send, no clear/inc race
                                nc.gpsimd.sem_clear(prep)
                                nc.gpsimd.sem_clear(rsem)
                                nc.gpsimd.sem_clear(lsem)
                    else:
                        nc.vector.tensor_copy(aout_peer[:], aout[:])

                    for qb in range(NTB):
                        for ec in range(NCC // 2):
                            f_ps = ps_a.tile([P, TB], f32, tag="sps",
                                             name=f"f_{qb}_{ec}")
                            for cc in range(NCC // 2):
                                nc.tensor.matmul(
                                    f_ps[:],
                                    woutT[:, cc, ec * P:(ec + 1) * P],
                                    aout[:, qb, cc, :],
                                    start=(cc == 0), stop=False)
                            for cc in range(NCC // 2):
                                nc.tensor.matmul(
                                    f_ps[:],
                                    woutT[:, NCC // 2 + cc,
                                          ec * P:(ec + 1) * P],
                                    aout_peer[:, qb, cc, :],
                                    start=False, stop=(cc == NCC // 2 - 1),
                                    skip_group_check=True)
                            fsb = mscp.tile([P, TB], bf16, tag="fsb")
                            nc.vector.tensor_copy(fsb[:], f_ps[:])
                            nc.sync.dma_start(
                                out_d[ec * P:(ec + 1) * P,
                                      qb * TB:(qb + 1) * TB], fsb[:])

            if reps == 1:
                body()
            else:
                with tc.For_i(0, reps, 1):
                    body()

    nc.compile()
    return nc


def _host_inputs(x, W_qkv, W_out):
    """Per-core input dicts (bf16)."""
    from ml_dtypes import bfloat16

    x = np.ascontiguousarray(np.asarray(x, dtype=np.float32))
    W_qkv = np.asarray(W_qkv, dtype=np.float32)
    W_out = np.asarray(W_out, dtype=np.float32)

    # rope tables, transposed layout, 2-head stack
    inv = 1.0 / (10000.0 ** (np.arange(0, Dh, 2, dtype=np.float64) / Dh))
    ang = np.outer(np.arange(T, dtype=np.float64), inv)        # (T, 32)
    emb = np.concatenate([ang, ang], axis=1)                   # (T, 64)
    cosT = np.cos(emb).astype(np.float32).T                    # (64, T)
    sinT = np.sin(emb).astype(np.float32).T
    cos2 = np.ascontiguousarray(np.concatenate([cosT, cosT], 0))  # (128, T)
    sin2 = np.ascontiguousarray(np.concatenate([sinT, sinT], 0))

    # rotation matrix: rot(q) = R @ q ; lhsT = R2.T
    R = np.zeros((Dh, Dh), np.float32)
    for d in range(Dh // 2):
        R[d, d + Dh // 2] = -1.0
        R[d + Dh // 2, d] = 1.0
    R2 = np.zeros((P, P), np.float32)
    R2[:Dh, :Dh] = R
    R2[Dh:, Dh:] = R
    r2T = np.ascontiguousarray(R2.T)

    # additive causal mask for the diagonal 128-block, scores^T layout:
    # (I.T @ mneg)[k, j] = mneg[k, j] = -240 where j < k (q before k)
    mneg = np.where(np.arange(P)[None, :] < np.arange(P)[:, None],
                    -240.0, 0.0).astype(np.float32)
    ident = np.eye(P, dtype=np.float32)

    bf = lambda a: np.ascontiguousarray(a).astype(bfloat16)

    ins = []
    for c in range(N_CORES):
        b, tp = c // 2, c % 2
        heads = range(8 * tp, 8 * tp + 8)
        wq = np.concatenate([W_qkv[64 * h: 64 * h + 64] for h in heads], 0)
        wk = np.concatenate(
            [W_qkv[D + 64 * h: D + 64 * h + 64] for h in heads], 0)
        wv = np.concatenate(
            [W_qkv[2 * D + 64 * h: 2 * D + 64 * h + 64] for h in heads], 0)
        wqkT = np.concatenate([wq, wk], 0).T          # (1024, 1024)
        wvT = wv.T                                    # (1024, 512)
        # out-proj: rows = contraction features ordered [my heads; peer
        # heads], cols = my 512 output features
        my_rows = W_out.T[512 * tp: 512 * tp + 512,
                          512 * tp: 512 * tp + 512]
        peer_rows = W_out.T[512 * (1 - tp): 512 * (1 - tp) + 512,
                            512 * tp: 512 * tp + 512]
        woutT = np.concatenate([my_rows, peer_rows], 0)   # (1024, 512)
        ins.append({
            "xT": bf(x[b].T),
            "wqkT": bf(wqkT), "wvT": bf(wvT), "woutT": bf(woutT),
            "r2T": bf(r2T), "cos2": bf(cos2), "sin2": bf(sin2),
            "mneg": bf(mneg), "ident": bf(ident),
        })
    return ins


def kernel(x, W_qkv, W_out):
    from concourse.bass_utils import run_bass_kernel_spmd

    if "nc" not in _CACHE:
        _CACHE["nc"] = _build_program()
    nc = _CACHE["nc"]
    ins = _host_inputs(x, W_qkv, W_out)
    res = run_bass_kernel_spmd(nc, ins, list(range(N_CORES)))
    out = np.empty((B, T, D), dtype=np.float32)
    for c in range(N_CORES):
        b, tp = c // 2, c % 2
        out[b, :, 512 * tp: 512 * tp + 512] = \
            np.asarray(res.results[c]["out"]).astype(np.float32).T
    return out


# revision 5
# speedup vs baseline: 1.0644x; 1.0644x over previous
"""Tensor-parallel causal attention layer (RoPE) for 8 Trainium2 NeuronCores.

Problem: nn_AttentionTier (B=4, T=2048, D=1024, H=16, Dh=64), fp32 I/O.

Sharding: DP=4 over batch x TP=2 over heads (8 heads per core).
  core c -> batch c//2, head group c%2 (heads 8*(c%2) .. 8*(c%2)+8).

v2 design:
  - All on-chip tensors bf16 (host pre-casts inputs): half DMA traffic, DVE
    2x perf mode, no fp32r small-moving matmul penalty.
  - Projection (PE-heavy, ACT-idle) and attention (ACT-heavy) INTERLEAVED
    per 512-token block: proj(tb) ; attn(qb=tb). The softmax exp stream for
    block qb overlaps the projection matmuls for block tb=qb+1.
  - Causal mask applied INSIDE the PE accumulation: after the diagonal score
    matmul, a second matmul (identity lhsT x (-240 strict-upper) rhs)
    accumulates -240 into masked entries, so exp(0.125*s) underflows to 0.
    No DVE op between exp and the AV matmul.
  - The per-qb out-proj ReduceScatter collective of the baseline is replaced
    by ONE SBUF->SBUF pairwise exchange of normalized attention outputs
    (remote_dma_broadcast, relative dest (0,1) = pair core), then each core
    computes the full out-projection for its own 512 output features over
    all 16 heads. woutT rows host-reordered [my feats; peer feats].
  - Softmax denominators: o_ps row 64 (ones-augmented V) -> per-qb [65,H,TB]
    evac tile; single DMA gathers all 8 sumexp rows (bf16->f32 cast) into a
    [1, H*TB] stack; one DVE reciprocal + bf16 cast; per-head K=1 matmul
    broadcasts recips over 64 partitions; one DVE mult normalizes.
  - Weights DMA'd before x so the PE starts ~immediately.
"""

import sys

sys.path.insert(0, "/opt/trn_rl_repo")

import numpy as np

B, T, D = 4, 2048, 1024
H, Dh = 16, 64
N_CORES = 8
P = 128
TB = 512          # token block (matmul moving dim)
NTB = T // TB     # 4
NCC = D // P      # 8 contraction chunks
HLOC = H // 2     # heads per core

_CACHE = {}


def _build_program(reps=1, exch="rdma"):
    import concourse.bass as bass  # noqa: F401
    import concourse.mybir as mybir
    import concourse.tile as tile
    from concourse import bacc

    f32 = mybir.dt.float32
    bf16 = mybir.dt.bfloat16
    AF = mybir.ActivationFunctionType

    nc = bacc.Bacc("TRN2", target_bir_lowering=False, debug=False,
                   num_devices=N_CORES)

    # ---- DRAM I/O (bf16 in/out; host casts) ----
    xT_d = nc.dram_tensor("xT", [D, T], bf16, kind="ExternalInput").ap()
    wqkT_d = nc.dram_tensor("wqkT", [D, D], bf16, kind="ExternalInput").ap()
    wvT_d = nc.dram_tensor("wvT", [D, D // 2], bf16, kind="ExternalInput").ap()
    woutT_d = nc.dram_tensor("woutT", [D, D // 2], bf16,
                             kind="ExternalInput").ap()
    r2T_d = nc.dram_tensor("r2T", [P, P], bf16, kind="ExternalInput").ap()
    cos2_d = nc.dram_tensor("cos2", [P, T], bf16, kind="ExternalInput").ap()
    sin2_d = nc.dram_tensor("sin2", [P, T], bf16, kind="ExternalInput").ap()
    mneg_d = nc.dram_tensor("mneg", [P, P], bf16, kind="ExternalInput").ap()
    ident_d = nc.dram_tensor("ident", [P, P], bf16, kind="ExternalInput").ap()
    out_d = nc.dram_tensor("out", [D // 2, T], bf16, kind="ExternalOutput").ap()

    # exchange semaphores (SPMD: same numbers on all cores)
    prep = nc.alloc_semaphore("prep")
    lsem = nc.alloc_semaphore("lsem")
    rsem = nc.alloc_semaphore("rsem")

    with tile.TileContext(nc) as tc:
        with tc.tile_pool(name="const", bufs=1) as constp, \
             tc.tile_pool(name="big", bufs=1) as bigp:

            r2T = constp.tile([P, P], bf16)
            nc.sync.dma_start(r2T[:], r2T_d[:])
            mneg = constp.tile([P, P], bf16)
            nc.sync.dma_start(mneg[:], mneg_d[:])
            ident = constp.tile([P, P], bf16)
            nc.sync.dma_start(ident[:], ident_d[:])
            ones_b = constp.tile([P, P], bf16)
            nc.vector.memset(ones_b[:], 1.0)

            # persistent big tensors
            qk = bigp.tile([P, NCC, T], bf16)                    # 32KB/p
            vbar = bigp.tile([P, T // P, HLOC, Dh + 1], bf16)    # ~17KB/p
            aout = bigp.tile([P, NTB, NCC // 2, TB], bf16)       # 16KB/p
            aout_peer = bigp.tile([P, NTB, NCC // 2, TB], bf16)  # 16KB/p
            partL = bigp.tile([P, 2, NCC // 2, TB], bf16)        # 8KB/p

            def body():
                with tc.tile_pool(name="w1", bufs=1) as w1p, \
                     tc.tile_pool(name="ph1", bufs=3) as ph1, \
                     tc.tile_pool(name="xtp", bufs=2) as xtp, \
                     tc.tile_pool(name="att", bufs=6) as attp, \
                     tc.tile_pool(name="msc", bufs=2) as mscp, \
                     tc.tile_pool(name="msc1", bufs=1) as mscp1, \
                     tc.tile_pool(name="ps_a", bufs=2, space="PSUM") as ps_a, \
                     tc.tile_pool(name="ps_b", bufs=2, space="PSUM") as ps_b:
                    # psum tags: "qs" qk-proj/rot (2 banks), "sps" scores
                    # (2x2 banks), "vob" v-proj/o/b (2 banks) => 8 banks
                    wqkT = w1p.tile([P, NCC, D], bf16)
                    wvT = w1p.tile([P, NCC, D // 2], bf16)
                    woutT = w1p.tile([P, NCC, D // 2], bf16)
                    cosb = w1p.tile([P, T], bf16)
                    sinb = w1p.tile([P, T], bf16)
                    # first weight chunk first, then x block 0, then the rest
                    nc.sync.dma_start(wqkT[:, 0], wqkT_d[0:P, :])

                    def load_xT(tb):
                        t = xtp.tile([P, NCC, TB], bf16, tag="xT")
                        for cc in range(NCC):
                            nc.sync.dma_start(
                                t[:, cc],
                                xT_d[cc * P:(cc + 1) * P,
                                     tb * TB:(tb + 1) * TB])
                        return t

                    xT0 = xtp.tile([P, NCC, TB], bf16, tag="xT")
                    nc.sync.dma_start(xT0[:, 0], xT_d[0:P, 0:TB])
                    for cc in range(1, NCC):
                        nc.sync.dma_start(
                            wqkT[:, cc], wqkT_d[cc * P:(cc + 1) * P, :])
                        nc.sync.dma_start(
                            xT0[:, cc], xT_d[cc * P:(cc + 1) * P, 0:TB])
                    for cc in range(NCC):
                        nc.sync.dma_start(
                            wvT[:, cc], wvT_d[cc * P:(cc + 1) * P, :])
                    nc.sync.dma_start(cosb[:], cos2_d[:])
                    nc.sync.dma_start(sinb[:], sin2_d[:])
                    nc.sync.dma_start(
                        woutT[:], woutT_d.rearrange("(cc p) o -> p cc o", p=P))

                    def rope_tail(oc, raw, tsl):
                        """rot matmul + cos/sin combine for chunk oc."""
                        rot_ps = ps_a.tile([P, TB], f32, tag="qs",
                                           name=f"rot_{oc}_{tsl.start}")
                        nc.tensor.matmul(rot_ps[:], r2T[:], raw[:],
                                         start=True, stop=True)
                        m1 = ph1.tile([P, TB], bf16, tag="m1")
                        nc.vector.tensor_tensor(
                            m1[:], raw[:], cosb[:, tsl],
                            mybir.AluOpType.mult)
                        m2 = ph1.tile([P, TB], bf16, tag="m2")
                        nc.vector.tensor_tensor(
                            m2[:], rot_ps[:], sinb[:, tsl],
                            mybir.AluOpType.mult)
                        nc.vector.tensor_tensor(
                            qk[:, oc, tsl], m1[:], m2[:],
                            mybir.AluOpType.add)

                    def proj(tb):
                        tsl = slice(tb * TB, (tb + 1) * TB)
                        xT = xT0 if tb == 0 else load_xT(tb)
                        pend = None
                        for oc in range(NCC):
                            qk_ps = ps_a.tile([P, TB], f32, tag="qs",
                                              name=f"qk_{oc}_{tb}")
                            for cc in range(NCC):
                                nc.tensor.matmul(
                                    qk_ps[:], wqkT[:, cc, oc * P:(oc + 1) * P],
                                    xT[:, cc, :],
                                    start=(cc == 0), stop=(cc == NCC - 1))
                            raw = ph1.tile([P, TB], bf16, tag="raw")
                            nc.scalar.activation(raw[:], qk_ps[:], AF.Copy)
                            if pend is not None:
                                rope_tail(pend[0], pend[1], tsl)
                            pend = (oc, raw)

                        # V projection (natural layout), rope tail of the
                        # last chunk slotted after the first V block
                        for ts in range(TB // P):
                            v_ps = ps_b.tile([P, D // 2], f32, tag="vob",
                                             name=f"v_{tb}_{ts}")
                            for cc in range(NCC):
                                nc.tensor.matmul(
                                    v_ps[:], xT[:, cc, ts * P:(ts + 1) * P],
                                    wvT[:, cc, :],
                                    start=(cc == 0), stop=(cc == NCC - 1))
                            tc_idx = tb * (TB // P) + ts
                            nc.vector.tensor_copy(
                                vbar[:, tc_idx, :, 0:Dh],
                                v_ps[:].rearrange("p (h d) -> p h d", h=HLOC))
                            if ts == 0:
                                rope_tail(pend[0], pend[1], tsl)
                                pend = None
                        # ones column for this tb's token chunks
                        nc.vector.tensor_copy(
                            vbar[:, 4 * tb:4 * tb + 4, :, Dh:Dh + 1],
                            ones_b[:, None, :HLOC, None].to_broadcast(
                                [P, 4, HLOC, 1]))

                    def attn(qb):
                        osball = mscp.tile([Dh + 1, HLOC, TB], bf16,
                                           tag="osball")
                        # sumexp rows live at partitions {0,32} x 4 cols so
                        # K=1 broadcast matmuls see 32-aligned bases.
                        # Gathered via HWDGE (bf16) to keep the SWDGE ring
                        # exclusively for the remote exchange.
                        sstack_b = mscp1.tile([P, 4, TB], bf16, tag="sstack_b")
                        for h in range(HLOC):
                            hb = Dh * (h % 2)
                            # q feats: chunks 0..3; k feats: chunks 4..7
                            qsl = (slice(hb, hb + Dh), h // 2,
                                   slice(qb * TB, (qb + 1) * TB))
                            ksl = lambda kc: qk[hb:hb + Dh, NCC // 2 + h // 2,
                                                kc * P:(kc + 1) * P]
                            o_ps = ps_b.tile([Dh + 1, TB], f32, tag="vob",
                                             name=f"o_{qb}_{h}")
                            # full (off-diagonal) k-chunks, two per exp
                            for kp in range(2 * qb):
                                k0 = 2 * kp
                                s_ps = ps_a.tile([P, 2, TB], f32, tag="sps")
                                nc.tensor.matmul(
                                    s_ps[:, 0, :], ksl(k0), qk[qsl],
                                    start=True, stop=True)
                                nc.tensor.matmul(
                                    s_ps[:, 1, :], ksl(k0 + 1), qk[qsl],
                                    start=True, stop=True)
                                pt = attp.tile([P, 2, TB], bf16, tag="pt")
                                nc.scalar.activation(
                                    pt[:], s_ps[:], AF.Exp, scale=0.125)
                                for j in range(2):
                                    nc.tensor.matmul(
                                        o_ps[:], vbar[:, k0 + j, h, :],
                                        pt[:, j, :],
                                        start=(k0 + j == 0), stop=False,
                                        skip_group_check=True)
                            # diagonal k-chunks, two per exp; causal mask
                            # folded into the PE accumulation (-240 on the
                            # strict upper triangle). The exp covers
                            # [qo0:TB] on both chunks -- the [qo0:qo1) cols
                            # of the second chunk are stale-PSUM garbage
                            # that the AV matmul never reads.
                            for crp in range(2):
                                kc0 = 4 * qb + 2 * crp
                                qo0 = 2 * crp * P
                                qo1 = qo0 + P
                                s_ps = ps_a.tile([P, 2, TB], f32, tag="sps")
                                nc.tensor.matmul(
                                    s_ps[:, 0, qo0:TB], ksl(kc0),
                                    qk[qsl][:, qo0:TB],
                                    start=True, stop=False)
                                nc.tensor.matmul(
                                    s_ps[:, 0, qo0:qo1], ident[:], mneg[:],
                                    start=False, stop=True,
                                    skip_group_check=True)
                                nc.tensor.matmul(
                                    s_ps[:, 1, qo1:TB], ksl(kc0 + 1),
                                    qk[qsl][:, qo1:TB],
                                    start=True, stop=False,
                                    skip_group_check=True)
                                nc.tensor.matmul(
                                    s_ps[:, 1, qo1:qo1 + P], ident[:],
                                    mneg[:], start=False, stop=True,
                                    skip_group_check=True)
                                pt = attp.tile([P, 2, TB], bf16, tag="pt")
                                nc.scalar.activation(
                                    pt[:, :, qo0:TB], s_ps[:, :, qo0:TB],
                                    AF.Exp, scale=0.125)
                                nc.tensor.matmul(
                                    o_ps[:, qo0:TB], vbar[:, kc0, h, :],
                                    pt[:, 0, qo0:TB],
                                    start=(kc0 == 0), stop=False,
                                    skip_group_check=True)
                                nc.tensor.matmul(
                                    o_ps[:, qo1:TB], vbar[:, kc0 + 1, h, :],
                                    pt[:, 1, qo1:TB],
                                    start=False, stop=(kc0 == 4 * qb + 2),
                                    skip_group_check=True)
                            # evacuate o_ps (incl. sumexp row 64) to bf16
                            nc.vector.tensor_copy(osball[:, h, :], o_ps[:])
                        # gather sumexp rows: head h lands at partition
                        # 32*(h//4), col h%4
                        for g in range(2):
                            nc.sync.dma_start(
                                sstack_b[32 * g:32 * g + 1, :, :],
                                osball[Dh:Dh + 1, 4 * g:4 * g + 4, :])
                        sstack = mscp1.tile([P, 4, TB], f32, tag="sstack")
                        rstack = mscp.tile([P, 4, TB], bf16, tag="rstack")
                        nc.vector.tensor_copy(sstack[:], sstack_b[:])
                        nc.vector.reciprocal(sstack[:], sstack[:])
                        nc.gpsimd.tensor_copy(rstack[:], sstack[:])
                        return osball, rstack

                    def finish_attn(qb, osball, rstack):
                        """Recip broadcast + normalize; emitted after the
                        NEXT proj block so the b_ps matmuls never stall the
                        PE queue on the reciprocal chain."""
                        for h in range(HLOC):
                            hb = Dh * (h % 2)
                            rrow = 32 * (h // 4)
                            b_ps = ps_b.tile([Dh + 1, TB], f32, tag="vob",
                                             name=f"b_{qb}_{h}")[0:Dh]
                            nc.tensor.matmul(
                                b_ps[:], ones_b[rrow:rrow + 1, 0:Dh],
                                rstack[rrow:rrow + 1, h % 4, :],
                                start=True, stop=True)
                            nc.vector.tensor_tensor(
                                aout[hb:hb + Dh, qb, h // 2, :],
                                osball[0:Dh, h, :], b_ps[:],
                                mybir.AluOpType.mult)

                    def part_local(qb):
                        """Local-half out-proj for qb, evacuated to partL;
                        emitted into the ACT-bound attention stretch where
                        the PE would otherwise idle."""
                        for ec in range(NCC // 2):
                            f_ps = ps_a.tile([P, TB], f32, tag="sps",
                                             name=f"pl_{qb}_{ec}")
                            for cc in range(NCC // 2):
                                nc.tensor.matmul(
                                    f_ps[:],
                                    woutT[:, cc, ec * P:(ec + 1) * P],
                                    aout[:, qb, cc, :],
                                    start=(cc == 0), stop=(cc == NCC // 2 - 1))
                            nc.vector.tensor_copy(partL[:, qb, ec, :], f_ps[:])

                    pend_fin = None
                    for tb in range(NTB):
                        proj(tb)
                        if pend_fin is not None:
                            finish_attn(*pend_fin)
                        if tb == NTB - 1:
                            part_local(0)
                        pend_fin = (tb,) + attn(tb)
                    part_local(1)
                    finish_attn(*pend_fin)

                    # ======== exchange + out-projection ========
                    if exch == "rdma":
                        # NB: tried moving the per-qb sends earlier (descgen+
                        # trigger right after each finish_attn, outside the
                        # critical, wait-only critical here) -- hangs the NRT.
                        # The whole exchange must stay inside one critical.
                        with tc.tile_critical():
                            for i in range(NTB):
                                nc.gpsimd.remote_dma_broadcast(
                                    aout_peer[:, i], aout[:, i], rsem, lsem,
                                    rdests=[(0, 1)] * 8).then_inc(prep, 1)
                            nc.gpsimd.wait_ge(prep, NTB)
                            nc.gpsimd.trigger_dma(NTB)
                            nc.gpsimd.wait_ge(rsem, 16 * NTB)
                            nc.gpsimd.wait_ge(lsem, 16 * NTB)
                            if reps > 1:
                                # reset for the next rep; peer is >200us from
                                # its next send, no clear/inc race
                                nc.gpsimd.sem_clear(prep)
                                nc.gpsimd.sem_clear(rsem)
                                nc.gpsimd.sem_clear(lsem)
                    else:
                        nc.vector.tensor_copy(aout_peer[:], aout[:])

                    for qb in range(NTB):
                        for ec in range(NCC // 2):
                            f_ps = ps_a.tile([P, TB], f32, tag="sps",
                                             name=f"f_{qb}_{ec}")
                            if qb >= 2:
                                for cc in range(NCC // 2):
                                    nc.tensor.matmul(
                                        f_ps[:],
                                        woutT[:, cc, ec * P:(ec + 1) * P],
                                        aout[:, qb, cc, :],
                                        start=(cc == 0), stop=False)
                            for cc in range(NCC // 2):
                                nc.tensor.matmul(
                                    f_ps[:],
                                    woutT[:, NCC // 2 + cc,
                                          ec * P:(ec + 1) * P],
                                    aout_peer[:, qb, cc, :],
                                    start=(qb < 2 and cc == 0),
                                    stop=(cc == NCC // 2 - 1),
                                    skip_group_check=True)
                            fsb = mscp.tile([P, TB], bf16, tag="fsb")
                            if qb < 2:
                                # add the pre-computed local half
                                nc.vector.tensor_tensor(
                                    fsb[:], partL[:, qb, ec, :], f_ps[:],
                                    mybir.AluOpType.add)
                            else:
                                nc.vector.tensor_copy(fsb[:], f_ps[:])
                            nc.sync.dma_start(
                                out_d[ec * P:(ec + 1) * P,
                                      qb * TB:(qb + 1) * TB], fsb[:])

            if reps == 1:
                body()
            else:
                with tc.For_i(0, reps, 1):
                    body()

    nc.compile()
    return nc


def _host_inputs(x, W_qkv, W_out):
    """Per-core input dicts (bf16)."""
    from ml_dtypes import bfloat16

    x = np.ascontiguousarray(np.asarray(x, dtype=np.float32))
    W_qkv = np.asarray(W_qkv, dtype=np.float32)
    W_out = np.asarray(W_out, dtype=np.float32)

    # rope tables, transposed layout, 2-head stack
    inv = 1.0 / (10000.0 ** (np.arange(0, Dh, 2, dtype=np.float64) / Dh))
    ang = np.outer(np.arange(T, dtype=np.float64), inv)        # (T, 32)
    emb = np.concatenate([ang, ang], axis=1)                   # (T, 64)
    cosT = np.cos(emb).astype(np.float32).T                    # (64, T)
    sinT = np.sin(emb).astype(np.float32).T
    cos2 = np.ascontiguousarray(np.concatenate([cosT, cosT], 0))  # (128, T)
    sin2 = np.ascontiguousarray(np.concatenate([sinT, sinT], 0))

    # rotation matrix: rot(q) = R @ q ; lhsT = R2.T
    R = np.zeros((Dh, Dh), np.float32)
    for d in range(Dh // 2):
        R[d, d + Dh // 2] = -1.0
        R[d + Dh // 2, d] = 1.0
    R2 = np.zeros((P, P), np.float32)
    R2[:Dh, :Dh] = R
    R2[Dh:, Dh:] = R
    r2T = np.ascontiguousarray(R2.T)

    # additive causal mask for the diagonal 128-block, scores^T layout:
    # (I.T @ mneg)[k, j] = mneg[k, j] = -240 where j < k (q before k)
    mneg = np.where(np.arange(P)[None, :] < np.arange(P)[:, None],
                    -240.0, 0.0).astype(np.float32)
    ident = np.eye(P, dtype=np.float32)

    bf = lambda a: np.ascontiguousarray(a).astype(bfloat16)

    ins = []
    for c in range(N_CORES):
        b, tp = c // 2, c % 2
        heads = range(8 * tp, 8 * tp + 8)
        wq = np.concatenate([W_qkv[64 * h: 64 * h + 64] for h in heads], 0)
        wk = np.concatenate(
            [W_qkv[D + 64 * h: D + 64 * h + 64] for h in heads], 0)
        wv = np.concatenate(
            [W_qkv[2 * D + 64 * h: 2 * D + 64 * h + 64] for h in heads], 0)
        wqkT = np.concatenate([wq, wk], 0).T          # (1024, 1024)
        wvT = wv.T                                    # (1024, 512)
        # out-proj: rows = contraction features ordered [my heads; peer
        # heads], cols = my 512 output features
        my_rows = W_out.T[512 * tp: 512 * tp + 512,
                          512 * tp: 512 * tp + 512]
        peer_rows = W_out.T[512 * (1 - tp): 512 * (1 - tp) + 512,
                            512 * tp: 512 * tp + 512]
        woutT = np.concatenate([my_rows, peer_rows], 0)   # (1024, 512)
        ins.append({
            "xT": bf(x[b].T),
            "wqkT": bf(wqkT), "wvT": bf(wvT), "woutT": bf(woutT),
            "r2T": bf(r2T), "cos2": bf(cos2), "sin2": bf(sin2),
            "mneg": bf(mneg), "ident": bf(ident),
        })
    return ins


def kernel(x, W_qkv, W_out):
    from concourse.bass_utils import run_bass_kernel_spmd

    if "nc" not in _CACHE:
        _CACHE["nc"] = _build_program()
    nc = _CACHE["nc"]
    ins = _host_inputs(x, W_qkv, W_out)
    res = run_bass_kernel_spmd(nc, ins, list(range(N_CORES)))
    out = np.empty((B, T, D), dtype=np.float32)
    for c in range(N_CORES):
        b, tp = c // 2, c % 2
        out[b, :, 512 * tp: 512 * tp + 512] = \
            np.asarray(res.results[c]["out"]).astype(np.float32).T
    return out


# revision 6
# speedup vs baseline: 2.5149x; 2.3627x over previous
"""Tensor-parallel causal attention layer (RoPE) for 8 Trainium2 NeuronCores.

Problem: nn_AttentionTier (B=4, T=2048, D=1024, H=16, Dh=64), fp32 I/O.

Sharding: DP=4 over batch x TP=2 over heads (8 heads per core).
  core c -> batch c//2, head group c%2 (heads 8*(c%2) .. 8*(c%2)+8).

v2 design:
  - All on-chip tensors bf16 (host pre-casts inputs): half DMA traffic, DVE
    2x perf mode, no fp32r small-moving matmul penalty.
  - Projection (PE-heavy, ACT-idle) and attention (ACT-heavy) INTERLEAVED
    per 512-token block: proj(tb) ; attn(qb=tb). The softmax exp stream for
    block qb overlaps the projection matmuls for block tb=qb+1.
  - Causal mask applied INSIDE the PE accumulation: after the diagonal score
    matmul, a second matmul (identity lhsT x (-240 strict-upper) rhs)
    accumulates -240 into masked entries, so exp(0.125*s) underflows to 0.
    No DVE op between exp and the AV matmul.
  - The per-qb out-proj ReduceScatter collective of the baseline is replaced
    by ONE SBUF->SBUF pairwise exchange of normalized attention outputs
    (remote_dma_broadcast, relative dest (0,1) = pair core), then each core
    computes the full out-projection for its own 512 output features over
    all 16 heads. woutT rows host-reordered [my feats; peer feats].
  - Softmax denominators: o_ps row 64 (ones-augmented V) -> per-qb [65,H,TB]
    evac tile; single DMA gathers all 8 sumexp rows (bf16->f32 cast) into a
    [1, H*TB] stack; one DVE reciprocal + bf16 cast; per-head K=1 matmul
    broadcasts recips over 64 partitions; one DVE mult normalizes.
  - Weights DMA'd before x so the PE starts ~immediately.
"""

import sys

sys.path.insert(0, "/opt/trn_rl_repo")

import numpy as np

B, T, D = 4, 2048, 1024
H, Dh = 16, 64
N_CORES = 8
P = 128
TB = 512          # token block (matmul moving dim)
NTB = T // TB     # 4
NCC = D // P      # 8 contraction chunks
HLOC = H // 2     # heads per core

_CACHE = {}


def _build_program(reps=1, exch="rdma"):
    import concourse.bass as bass  # noqa: F401
    import concourse.mybir as mybir
    import concourse.tile as tile
    from concourse import bacc

    f32 = mybir.dt.float32
    bf16 = mybir.dt.bfloat16
    AF = mybir.ActivationFunctionType

    nc = bacc.Bacc("TRN2", target_bir_lowering=False, debug=False,
                   num_devices=N_CORES)

    # ---- DRAM I/O (bf16 in/out; host casts) ----
    xT_d = nc.dram_tensor("xT", [D, T], bf16, kind="ExternalInput").ap()
    wqkT_d = nc.dram_tensor("wqkT", [D, D], bf16, kind="ExternalInput").ap()
    wvT_d = nc.dram_tensor("wvT", [D, D // 2], bf16, kind="ExternalInput").ap()
    woutT_d = nc.dram_tensor("woutT", [D, D // 2], bf16,
                             kind="ExternalInput").ap()
    r2T_d = nc.dram_tensor("r2T", [P, P], bf16, kind="ExternalInput").ap()
    cos2_d = nc.dram_tensor("cos2", [P, T], bf16, kind="ExternalInput").ap()
    sin2_d = nc.dram_tensor("sin2", [P, T], bf16, kind="ExternalInput").ap()
    mneg_d = nc.dram_tensor("mneg", [P, P], bf16, kind="ExternalInput").ap()
    ident_d = nc.dram_tensor("ident", [P, P], bf16, kind="ExternalInput").ap()
    out_d = nc.dram_tensor("out", [D // 2, T], bf16, kind="ExternalOutput").ap()

    # exchange semaphores (SPMD: same numbers on all cores)
    prep = nc.alloc_semaphore("prep")
    lsem = nc.alloc_semaphore("lsem")
    rsem = nc.alloc_semaphore("rsem")

    with tile.TileContext(nc) as tc:
        with tc.tile_pool(name="const", bufs=1) as constp, \
             tc.tile_pool(name="big", bufs=1) as bigp, \
             tc.tile_pool(name="w1", bufs=1) as w1p, \
             tc.tile_pool(name="ph1", bufs=3) as ph1, \
             tc.tile_pool(name="xtp", bufs=2) as xtp, \
             tc.tile_pool(name="att", bufs=6) as attp, \
             tc.tile_pool(name="msc", bufs=2) as mscp, \
             tc.tile_pool(name="msc1", bufs=1) as mscp1, \
             tc.tile_pool(name="ps_a", bufs=2, space="PSUM") as ps_a, \
             tc.tile_pool(name="ps_b", bufs=2, space="PSUM") as ps_b:

            r2T = constp.tile([P, P], bf16)
            nc.sync.dma_start(r2T[:], r2T_d[:])
            mneg = constp.tile([P, P], bf16)
            nc.sync.dma_start(mneg[:], mneg_d[:])
            ident = constp.tile([P, P], bf16)
            nc.sync.dma_start(ident[:], ident_d[:])
            ones_b = constp.tile([P, P], bf16)
            nc.vector.memset(ones_b[:], 1.0)

            # persistent big tensors
            qk = bigp.tile([P, NCC, T], bf16)                    # 32KB/p
            vbar = bigp.tile([P, T // P, HLOC, Dh + 1], bf16)    # ~17KB/p
            aout = bigp.tile([P, NTB, NCC // 2, TB], bf16)       # 16KB/p
            aout_peer = bigp.tile([P, NTB, NCC // 2, TB], bf16)  # 16KB/p
            partL = bigp.tile([P, 2, NCC // 2, TB], bf16)        # 8KB/p

            # psum tags: "qs" qk-proj/rot (2 banks), "sps" scores
            # (2x2 banks), "vob" v-proj/o/b (2 banks) => 8 banks
            # Weights/tables loaded ONCE (outside the reps loop); the x
            # loads ride the ACT HWDGE queue so they don't serialize
            # behind the weight stream on the SP queue.
            wqkT = w1p.tile([P, NCC, D], bf16)
            wvT = w1p.tile([P, NCC, D // 2], bf16)
            woutT = w1p.tile([P, NCC, D // 2], bf16)
            cosb = w1p.tile([P, T], bf16)
            sinb = w1p.tile([P, T], bf16)
            for cc in range(NCC):
                nc.sync.dma_start(
                    wqkT[:, cc], wqkT_d[cc * P:(cc + 1) * P, :])
            for cc in range(NCC):
                nc.sync.dma_start(
                    wvT[:, cc], wvT_d[cc * P:(cc + 1) * P, :])
            nc.sync.dma_start(cosb[:], cos2_d[:])
            nc.sync.dma_start(sinb[:], sin2_d[:])
            nc.sync.dma_start(
                woutT[:], woutT_d.rearrange("(cc p) o -> p cc o", p=P))

            def body():
                if True:
                    def load_xT(tb):
                        t = xtp.tile([P, NCC, TB], bf16, tag="xT")
                        for cc in range(NCC):
                            nc.scalar.dma_start(
                                t[:, cc],
                                xT_d[cc * P:(cc + 1) * P,
                                     tb * TB:(tb + 1) * TB])
                        return t

                    xT0 = load_xT(0)

                    def rope_tail(oc, raw, tsl):
                        """rot matmul + cos/sin combine for chunk oc."""
                        rot_ps = ps_a.tile([P, TB], f32, tag="qs",
                                           name=f"rot_{oc}_{tsl.start}")
                        nc.tensor.matmul(rot_ps[:], r2T[:], raw[:],
                                         start=True, stop=True)
                        m1 = ph1.tile([P, TB], bf16, tag="m1")
                        nc.vector.tensor_tensor(
                            m1[:], raw[:], cosb[:, tsl],
                            mybir.AluOpType.mult)
                        m2 = ph1.tile([P, TB], bf16, tag="m2")
                        nc.vector.tensor_tensor(
                            m2[:], rot_ps[:], sinb[:, tsl],
                            mybir.AluOpType.mult)
                        nc.vector.tensor_tensor(
                            qk[:, oc, tsl], m1[:], m2[:],
                            mybir.AluOpType.add)

                    def proj(tb):
                        tsl = slice(tb * TB, (tb + 1) * TB)
                        xT = xT0 if tb == 0 else load_xT(tb)
                        pend = None
                        for oc in range(NCC):
                            qk_ps = ps_a.tile([P, TB], f32, tag="qs",
                                              name=f"qk_{oc}_{tb}")
                            for cc in range(NCC):
                                nc.tensor.matmul(
                                    qk_ps[:], wqkT[:, cc, oc * P:(oc + 1) * P],
                                    xT[:, cc, :],
                                    start=(cc == 0), stop=(cc == NCC - 1))
                            raw = ph1.tile([P, TB], bf16, tag="raw")
                            nc.scalar.activation(raw[:], qk_ps[:], AF.Copy)
                            if pend is not None:
                                rope_tail(pend[0], pend[1], tsl)
                            pend = (oc, raw)

                        # V projection (natural layout), rope tail of the
                        # last chunk slotted after the first V block
                        for ts in range(TB // P):
                            v_ps = ps_b.tile([P, D // 2], f32, tag="vob",
                                             name=f"v_{tb}_{ts}")
                            for cc in range(NCC):
                                nc.tensor.matmul(
                                    v_ps[:], xT[:, cc, ts * P:(ts + 1) * P],
                                    wvT[:, cc, :],
                                    start=(cc == 0), stop=(cc == NCC - 1))
                            tc_idx = tb * (TB // P) + ts
                            nc.vector.tensor_copy(
                                vbar[:, tc_idx, :, 0:Dh],
                                v_ps[:].rearrange("p (h d) -> p h d", h=HLOC))
                            if ts == 0:
                                rope_tail(pend[0], pend[1], tsl)
                                pend = None
                        # ones column for this tb's token chunks
                        nc.vector.tensor_copy(
                            vbar[:, 4 * tb:4 * tb + 4, :, Dh:Dh + 1],
                            ones_b[:, None, :HLOC, None].to_broadcast(
                                [P, 4, HLOC, 1]))

                    def attn(qb):
                        osball = mscp.tile([Dh + 1, HLOC, TB], bf16,
                                           tag="osball")
                        # sumexp rows live at partitions {0,32} x 4 cols so
                        # K=1 broadcast matmuls see 32-aligned bases.
                        # Gathered via HWDGE (bf16) to keep the SWDGE ring
                        # exclusively for the remote exchange.
                        sstack_b = mscp1.tile([P, 4, TB], bf16, tag="sstack_b")
                        for h in range(HLOC):
                            hb = Dh * (h % 2)
                            # q feats: chunks 0..3; k feats: chunks 4..7
                            qsl = (slice(hb, hb + Dh), h // 2,
                                   slice(qb * TB, (qb + 1) * TB))
                            ksl = lambda kc: qk[hb:hb + Dh, NCC // 2 + h // 2,
                                                kc * P:(kc + 1) * P]
                            o_ps = ps_b.tile([Dh + 1, TB], f32, tag="vob",
                                             name=f"o_{qb}_{h}")
                            # full (off-diagonal) k-chunks, two per exp
                            for kp in range(2 * qb):
                                k0 = 2 * kp
                                s_ps = ps_a.tile([P, 2, TB], f32, tag="sps")
                                nc.tensor.matmul(
                                    s_ps[:, 0, :], ksl(k0), qk[qsl],
                                    start=True, stop=True)
                                nc.tensor.matmul(
                                    s_ps[:, 1, :], ksl(k0 + 1), qk[qsl],
                                    start=True, stop=True)
                                pt = attp.tile([P, 2, TB], bf16, tag="pt")
                                nc.scalar.activation(
                                    pt[:], s_ps[:], AF.Exp, scale=0.125)
                                for j in range(2):
                                    nc.tensor.matmul(
                                        o_ps[:], vbar[:, k0 + j, h, :],
                                        pt[:, j, :],
                                        start=(k0 + j == 0), stop=False,
                                        skip_group_check=True)
                            # diagonal k-chunks, two per exp; causal mask
                            # folded into the PE accumulation (-240 on the
                            # strict upper triangle). The exp covers
                            # [qo0:TB] on both chunks -- the [qo0:qo1) cols
                            # of the second chunk are stale-PSUM garbage
                            # that the AV matmul never reads.
                            for crp in range(2):
                                kc0 = 4 * qb + 2 * crp
                                qo0 = 2 * crp * P
                                qo1 = qo0 + P
                                s_ps = ps_a.tile([P, 2, TB], f32, tag="sps")
                                nc.tensor.matmul(
                                    s_ps[:, 0, qo0:TB], ksl(kc0),
                                    qk[qsl][:, qo0:TB],
                                    start=True, stop=False)
                                nc.tensor.matmul(
                                    s_ps[:, 0, qo0:qo1], ident[:], mneg[:],
                                    start=False, stop=True,
                                    skip_group_check=True)
                                nc.tensor.matmul(
                                    s_ps[:, 1, qo1:TB], ksl(kc0 + 1),
                                    qk[qsl][:, qo1:TB],
                                    start=True, stop=False,
                                    skip_group_check=True)
                                nc.tensor.matmul(
                                    s_ps[:, 1, qo1:qo1 + P], ident[:],
                                    mneg[:], start=False, stop=True,
                                    skip_group_check=True)
                                pt = attp.tile([P, 2, TB], bf16, tag="pt")
                                nc.scalar.activation(
                                    pt[:, :, qo0:TB], s_ps[:, :, qo0:TB],
                                    AF.Exp, scale=0.125)
                                nc.tensor.matmul(
                                    o_ps[:, qo0:TB], vbar[:, kc0, h, :],
                                    pt[:, 0, qo0:TB],
                                    start=(kc0 == 0), stop=False,
                                    skip_group_check=True)
                                nc.tensor.matmul(
                                    o_ps[:, qo1:TB], vbar[:, kc0 + 1, h, :],
                                    pt[:, 1, qo1:TB],
                                    start=False, stop=(kc0 == 4 * qb + 2),
                                    skip_group_check=True)
                            # evacuate o_ps (incl. sumexp row 64) to bf16
                            nc.vector.tensor_copy(osball[:, h, :], o_ps[:])
                        # gather sumexp rows: head h lands at partition
                        # 32*(h//4), col h%4
                        for g in range(2):
                            nc.sync.dma_start(
                                sstack_b[32 * g:32 * g + 1, :, :],
                                osball[Dh:Dh + 1, 4 * g:4 * g + 4, :])
                        sstack = mscp1.tile([P, 4, TB], f32, tag="sstack")
                        rstack = mscp.tile([P, 4, TB], bf16, tag="rstack")
                        nc.vector.tensor_copy(sstack[:], sstack_b[:])
                        nc.vector.reciprocal(sstack[:], sstack[:])
                        nc.gpsimd.tensor_copy(rstack[:], sstack[:])
                        return osball, rstack

                    def finish_attn(qb, osball, rstack):
                        """Recip broadcast + normalize; emitted after the
                        NEXT proj block so the b_ps matmuls never stall the
                        PE queue on the reciprocal chain."""
                        for h in range(HLOC):
                            hb = Dh * (h % 2)
                            rrow = 32 * (h // 4)
                            b_ps = ps_b.tile([Dh + 1, TB], f32, tag="vob",
                                             name=f"b_{qb}_{h}")[0:Dh]
                            nc.tensor.matmul(
                                b_ps[:], ones_b[rrow:rrow + 1, 0:Dh],
                                rstack[rrow:rrow + 1, h % 4, :],
                                start=True, stop=True)
                            nc.vector.tensor_tensor(
                                aout[hb:hb + Dh, qb, h // 2, :],
                                osball[0:Dh, h, :], b_ps[:],
                                mybir.AluOpType.mult)

                    def part_local(qb):
                        """Local-half out-proj for qb, evacuated to partL;
                        emitted into the ACT-bound attention stretch where
                        the PE would otherwise idle."""
                        for ec in range(NCC // 2):
                            f_ps = ps_a.tile([P, TB], f32, tag="sps",
                                             name=f"pl_{qb}_{ec}")
                            for cc in range(NCC // 2):
                                nc.tensor.matmul(
                                    f_ps[:],
                                    woutT[:, cc, ec * P:(ec + 1) * P],
                                    aout[:, qb, cc, :],
                                    start=(cc == 0), stop=(cc == NCC // 2 - 1))
                            nc.vector.tensor_copy(partL[:, qb, ec, :], f_ps[:])

                    pend_fin = None
                    for tb in range(NTB):
                        proj(tb)
                        if pend_fin is not None:
                            finish_attn(*pend_fin)
                        if tb == NTB - 1:
                            part_local(0)
                        pend_fin = (tb,) + attn(tb)
                    part_local(1)
                    finish_attn(*pend_fin)

                    # ======== exchange + out-projection ========
                    if exch == "rdma":
                        # NB: tried moving the per-qb sends earlier (descgen+
                        # trigger right after each finish_attn, outside the
                        # critical, wait-only critical here) -- hangs the NRT.
                        # The whole exchange must stay inside one critical.
                        with tc.tile_critical():
                            for i in range(NTB):
                                nc.gpsimd.remote_dma_broadcast(
                                    aout_peer[:, i], aout[:, i], rsem, lsem,
                                    rdests=[(0, 1)] * 8).then_inc(prep, 1)
                            nc.gpsimd.wait_ge(prep, NTB)
                            nc.gpsimd.trigger_dma(NTB)
                            nc.gpsimd.wait_ge(rsem, 16 * NTB)
                            nc.gpsimd.wait_ge(lsem, 16 * NTB)
                            if reps > 1:
                                # reset for the next rep; peer is >200us from
                                # its next send, no clear/inc race
                                nc.gpsimd.sem_clear(prep)
                                nc.gpsimd.sem_clear(rsem)
                                nc.gpsimd.sem_clear(lsem)
                    else:
                        nc.vector.tensor_copy(aout_peer[:], aout[:])

                    for qb in range(NTB):
                        for ec in range(NCC // 2):
                            f_ps = ps_a.tile([P, TB], f32, tag="sps",
                                             name=f"f_{qb}_{ec}")
                            if qb >= 2:
                                for cc in range(NCC // 2):
                                    nc.tensor.matmul(
                                        f_ps[:],
                                        woutT[:, cc, ec * P:(ec + 1) * P],
                                        aout[:, qb, cc, :],
                                        start=(cc == 0), stop=False)
                            for cc in range(NCC // 2):
                                nc.tensor.matmul(
                                    f_ps[:],
                                    woutT[:, NCC // 2 + cc,
                                          ec * P:(ec + 1) * P],
                                    aout_peer[:, qb, cc, :],
                                    start=(qb < 2 and cc == 0),
                                    stop=(cc == NCC // 2 - 1),
                                    skip_group_check=True)
                            fsb = mscp.tile([P, TB], bf16, tag="fsb")
                            if qb < 2:
                                # add the pre-computed local half
                                nc.vector.tensor_tensor(
                                    fsb[:], partL[:, qb, ec, :], f_ps[:],
                                    mybir.AluOpType.add)
                            else:
                                nc.vector.tensor_copy(fsb[:], f_ps[:])
                            nc.sync.dma_start(
                                out_d[ec * P:(ec + 1) * P,
                                      qb * TB:(qb + 1) * TB], fsb[:])

            if reps == 1:
                body()
            else:
                with tc.For_i(0, reps, 1):
                    body()

    nc.compile()
    return nc


def _host_inputs(x, W_qkv, W_out):
    """Per-core input dicts (bf16)."""
    from ml_dtypes import bfloat16

    x = np.ascontiguousarray(np.asarray(x, dtype=np.float32))
    W_qkv = np.asarray(W_qkv, dtype=np.float32)
    W_out = np.asarray(W_out, dtype=np.float32)

    # rope tables, transposed layout, 2-head stack
    inv = 1.0 / (10000.0 ** (np.arange(0, Dh, 2, dtype=np.float64) / Dh))
    ang = np.outer(np.arange(T, dtype=np.float64), inv)        # (T, 32)
    emb = np.concatenate([ang, ang], axis=1)                   # (T, 64)
    cosT = np.cos(emb).astype(np.float32).T                    # (64, T)
    sinT = np.sin(emb).astype(np.float32).T
    cos2 = np.ascontiguousarray(np.concatenate([cosT, cosT], 0))  # (128, T)
    sin2 = np.ascontiguousarray(np.concatenate([sinT, sinT], 0))

    # rotation matrix: rot(q) = R @ q ; lhsT = R2.T
    R = np.zeros((Dh, Dh), np.float32)
    for d in range(Dh // 2):
        R[d, d + Dh // 2] = -1.0
        R[d + Dh // 2, d] = 1.0
    R2 = np.zeros((P, P), np.float32)
    R2[:Dh, :Dh] = R
    R2[Dh:, Dh:] = R
    r2T = np.ascontiguousarray(R2.T)

    # additive causal mask for the diagonal 128-block, scores^T layout:
    # (I.T @ mneg)[k, j] = mneg[k, j] = -240 where j < k (q before k)
    mneg = np.where(np.arange(P)[None, :] < np.arange(P)[:, None],
                    -240.0, 0.0).astype(np.float32)
    ident = np.eye(P, dtype=np.float32)

    bf = lambda a: np.ascontiguousarray(a).astype(bfloat16)

    ins = []
    for c in range(N_CORES):
        b, tp = c // 2, c % 2
        heads = range(8 * tp, 8 * tp + 8)
        wq = np.concatenate([W_qkv[64 * h: 64 * h + 64] for h in heads], 0)
        wk = np.concatenate(
            [W_qkv[D + 64 * h: D + 64 * h + 64] for h in heads], 0)
        wv = np.concatenate(
            [W_qkv[2 * D + 64 * h: 2 * D + 64 * h + 64] for h in heads], 0)
        wqkT = np.concatenate([wq, wk], 0).T          # (1024, 1024)
        wvT = wv.T                                    # (1024, 512)
        # out-proj: rows = contraction features ordered [my heads; peer
        # heads], cols = my 512 output features
        my_rows = W_out.T[512 * tp: 512 * tp + 512,
                          512 * tp: 512 * tp + 512]
        peer_rows = W_out.T[512 * (1 - tp): 512 * (1 - tp) + 512,
                            512 * tp: 512 * tp + 512]
        woutT = np.concatenate([my_rows, peer_rows], 0)   # (1024, 512)
        ins.append({
            "xT": bf(x[b].T),
            "wqkT": bf(wqkT), "wvT": bf(wvT), "woutT": bf(woutT),
            "r2T": bf(r2T), "cos2": bf(cos2), "sin2": bf(sin2),
            "mneg": bf(mneg), "ident": bf(ident),
        })
    return ins


def kernel(x, W_qkv, W_out):
    from concourse.bass_utils import run_bass_kernel_spmd

    if "nc" not in _CACHE:
        _CACHE["nc"] = _build_program()
    nc = _CACHE["nc"]
    ins = _host_inputs(x, W_qkv, W_out)
    res = run_bass_kernel_spmd(nc, ins, list(range(N_CORES)))
    out = np.empty((B, T, D), dtype=np.float32)
    for c in range(N_CORES):
        b, tp = c // 2, c % 2
        out[b, :, 512 * tp: 512 * tp + 512] = \
            np.asarray(res.results[c]["out"]).astype(np.float32).T
    return out
